# revision 49
# baseline (speedup 1.0000x reference)
# Trainium2 Bass kernel for nn_AxonalConnections (gnn_message_passing).
#
# Computes out[B, H, W] = (spikes.reshape(B, N) @ adjacency.T).reshape(B, H, W)
# with B=16, H=W=128, N=16384 on 8 NeuronCores.
#
# Strategy (pure tensor parallelism, no collectives):
#   - Shard adjacency row-wise (target dim) across 8 cores: core i owns
#     target columns [i*2048, (i+1)*2048) of the output.
#   - The kernel is HBM/DMA-bandwidth bound, so minimize shipped bytes:
#     * input-adaptive block pruning: the host scans the adjacency at
#       [128 x 128] block granularity (source grid-row si x target grid-row
#       ti) and ships only blocks that contain nonzeros. For the conv-
#       structured adjacency this is ~112 of 2048 blocks per core (7.3 MiB
#       vs 128 MiB); for dense inputs every block ships and the kernel
#       stays exact. Per-core block sets are aligned by a per-core source
#       offset into one shared pattern so all 8 cores run the same NEFF.
#     * fp32 would stream 4x slower through the PE, so both operands ship
#       as fp16 hi/lo pairs (x = hi + lo, exact to ~2^-22): full fp32-grade
#       accuracy (~1e-6 output error) at full PE streaming rate.
#   - Spikes (tiny) are packed as the stationary operand
#     [spikes_hi | spikes_lo] (32 columns); the adjacency hi and (scaled)
#     lo block streams accumulate into separate PSUM bank sets. The host
#     folds the four partial terms and concatenates the target shards.
#   - Blocks stream si-major with merged matmuls over consecutive ti; each
#     PSUM bank finishes early and its PSUM->SBUF copy + output DMA overlap
#     the remaining matmuls.
#
# Single-queue HWDGE DMA with 8 KiB per-partition runs sustains ~410 GB/s
# (95% of the 435 GB/s SBUF-AXI fabric ceiling).

import numpy as np

B = 16
H = 128
W = 128
N = H * W            # 16384 source == target size
NCORES = 8
TSH = N // NCORES    # 2048 target columns per core
TI = TSH // W        # 16 target grid-rows per core
P = 128              # SBUF partitions / contraction tile
SCHUNKS = N // P     # 128 source chunks (== source grid-rows)
BLK_GROUP = 32       # blocks per DMA (32 * 32 KiB = 1 MiB, 8 KiB runs)
BLK = P * P          # elements per block

_cache = {}


N_WARM = 0  # PE warmup disabled: warmup MMs into a live accumulator bank crash the PE
LO_SCALE = 1024.0  # lo-residual pre-scale (keeps fp16 lo values in normal range)


def _plan_segments(pattern, group_sizes):
    """Plan merged matmuls over the si-major block stream.

    pattern: list of (ti, si_rel), si-major then ti-ascending — the stream
    order. Blocks with consecutive ti, the same source chunk, the same PSUM
    bank, and the same DMA group merge into one matmul of N = 128*len.

    start=True is set ONLY on the first segment of each PSUM bank: on HW it
    clears has_written for the WHOLE bank, and the per-element has_written
    bit then makes every region's first write an overwrite and later writes
    accumulates — no per-region start flags needed (a later start=True
    would wipe the has_written state of sibling regions mid-accumulation).

    Returns segments: list of (k0, nblk, si_rel, ti0, start).
    """
    group_of = []
    for g, gsz in enumerate(group_sizes):
        group_of += [g] * gsz
    segments = []
    k = 0
    n = len(pattern)
    seen_banks = set()
    while k < n:
        ti0, s = pattern[k]
        ln = 1
        while (
            k + ln < n
            and pattern[k + ln] == (ti0 + ln, s)
            and (ti0 + ln) // 4 == ti0 // 4
            and group_of[k + ln] == group_of[k]
        ):
            ln += 1
        bank = ti0 // 4
        segments.append((k, ln, s, ti0, bank not in seen_banks))
        seen_banks.add(bank)
        k += ln
    return segments


def _build_nc(pattern, n_spk):
    """Build + compile the SPMD Bass program.

    pattern: list of (ti, si_rel) block coordinates in si-major stream
             order, identical for all cores. Every ti in [0, TI) appears.
    n_spk:   number of stationary source chunks shipped (max si_rel + 1).
    """
    import concourse.mybir as mybir
    import concourse.tile as tile
    from concourse import bacc

    n_blocks = len(pattern)
    group_sizes = _group_sizes(n_blocks)
    segments = _plan_segments(pattern, group_sizes)

    nc = bacc.Bacc(
        "TRN2",
        target_bir_lowering=False,
        debug=False,
        num_devices=NCORES,
    )
    # ablk: flat stream of gathered [128 x 128] fp16 blocks in `pattern`
    # order, packed per DMA-group as [p, group_blocks*128] (partition-major)
    # so every descriptor moves one contiguous 8 KiB run per partition.
    # Two halves: blocks of fp16_hi(adj), then blocks of
    # (adj - fp16_hi(adj)) * LO_SCALE — together fp32-exact to ~2^-22.
    ablk = nc.dram_tensor(
        "ablk", [2 * n_blocks * BLK], mybir.dt.float16, kind="ExternalInput"
    ).ap()
    # spk: stationary weights for the shipped source-chunk window, packed
    # [P, n_spk*32] fp16 where
    #   spk[p, k*32 + b]      = fp16_hi(spikes[b, (o_i + k)*128 + p])
    #   spk[p, k*32 + 16 + b] = fp16_lo(spikes[b, (o_i + k)*128 + p])
    # (o_i = per-core source offset; out-of-range chunks are zero).
    spk = nc.dram_tensor(
        "spk", [P, n_spk * 32], mybir.dt.float16, kind="ExternalInput"
    ).ap()
    # Output: [hi-terms | lo-terms] per target shard; host folds
    # out = (hi[0:16]+hi[16:32]) + (lo[0:16]+lo[16:32])/LO_SCALE.
    out = nc.dram_tensor(
        "o", [32, 2 * TSH], mybir.dt.float32, kind="ExternalOutput"
    ).ap()

    f32 = mybir.dt.float32
    f16 = mybir.dt.float16
    NJ = 4  # PSUM banks ([32, 512] each; 4 ti-blocks per bank)

    # Last stream index per PSUM bank (drives the drain copies).
    last_k_bank = {}
    for k, (ti, _) in enumerate(pattern):
        last_k_bank[ti // NJ] = k

    # Map stream index -> (group, local index, group start offset).
    grp_of = []
    for g, gsz in enumerate(group_sizes):
        base = len(grp_of)
        grp_of += [(g, kk - base) for kk in range(base, base + gsz)]

    with tile.TileContext(nc) as tc:
        with (
            tc.tile_pool(name="adj", bufs=min(8, 2 * len(group_sizes))) as adj_pool,
            tc.tile_pool(name="spkp", bufs=1) as spk_pool,
            tc.tile_pool(name="warm", bufs=1) as warm_pool,
            tc.tile_pool(name="psum", bufs=1, space="PSUM") as psum_pool,
            tc.tile_pool(name="outp", bufs=1) as out_pool,
        ):
            ps = [
                psum_pool.tile([32, NJ * P], f32, name=f"ps{h}_{j}", tag=f"ps{h}_{j}")
                for h in range(2)
                for j in range(NJ)
            ]  # 8 banks: 0-3 hi, 4-7 lo

            # PE warmup: ~6 us of dummy matmuls on a zeroed tile, scheduled
            # before any real data arrives. They release the HAM clock gate
            # (cold PE runs at 1.2 GHz for the first ~3.4 us of activity) so
            # the real matmuls run at 2.4 GHz from the start. They target
            # the LAST lo bank, whose first real segment re-initializes it
            # (start=True) long after the warmup is done.
            dumt = warm_pool.tile([P, 512], f16)
            nc.gpsimd.memset(dumt[:], 0.0)
            for _ in range(N_WARM):
                nc.tensor.matmul(
                    ps[7][:, :],
                    dumt[:, 0:32],
                    dumt[:, :],
                    start=True,
                    stop=True,
                    skip_group_check=True,
                )

            # Stationary weights load first on the SP ring: every matmul
            # waits on them, and on the ACT ring their packets get
            # interleaved behind the block stream (first matmul slips by
            # ~2.5 us). Serializing ~0.5 us ahead of the stream is cheaper.
            spk_t = spk_pool.tile([P, n_spk * 32], f16)
            nc.sync.dma_start(spk_t[:], spk[:])

            ot = out_pool.tile([32, 2 * TSH], f32)

            for h in range(2):  # 0 = hi stream, 1 = lo stream
                at_tiles = []
                off = h * n_blocks * BLK
                for g, gsz in enumerate(group_sizes):
                    at = adj_pool.tile([P, gsz * P], f16, name=f"at{h}_{g}", tag="at")
                    nc.sync.dma_start(
                        at[:].rearrange("p (n t) -> p n t", n=gsz),
                        ablk[off : off + gsz * BLK].rearrange(
                            "(p n t) -> p n t", p=P, t=P
                        ),
                    )
                    off += gsz * BLK
                    at_tiles.append(at)

                for k0, nblk, si_rel, ti0, start in segments:
                    g, kl = grp_of[k0]
                    j, c = divmod(ti0, NJ)
                    pj = ps[h * NJ + j]
                    nc.tensor.matmul(
                        pj[:, c * P : (c + nblk) * P],
                        spk_t[:, si_rel * 32 : (si_rel + 1) * 32],
                        at_tiles[g][:, kl * P : (kl + nblk) * P],
                        start=start,
                        stop=(k0 + nblk - 1 == last_k_bank[j]),
                        skip_group_check=True,
                    )
                    if k0 + nblk - 1 == last_k_bank[j]:
                        # Bank fully accumulated: drain it while the
                        # remaining banks' matmuls keep streaming. The store
                        # goes on the ACT HWDGE ring — on the (in-order) SP
                        # ring its semaphore wait would block later DMA
                        # issues behind it.
                        sl = slice(
                            (h * NJ + j) * NJ * P, (h * NJ + j + 1) * NJ * P
                        )
                        nc.vector.tensor_copy(ot[:, sl], pj[:, :])
                        nc.scalar.dma_start(out[:, sl], ot[:, sl])

    nc.compile()
    return nc


def _group_sizes(n_blocks):
    """DMA group sizes: 1 MiB groups, tapering the tail so the last
    group's matmuls + completion latency (critical path) are short.

    (A head-taper was tried and is a net loss: the PE then tracks the DMA
    stream closely and pays each group's ~1.7 us completion-receipt
    latency as a stall; with deep prefetch and a late PE start the
    completion semaphores clear before the PE reaches them.)"""
    sizes = []
    rem = n_blocks
    while rem > BLK_GROUP:
        sizes.append(BLK_GROUP)
        rem -= BLK_GROUP
    while rem > 4:
        h = max(4, rem // 2)
        sizes.append(h)
        rem -= h
    if rem:
        sizes.append(rem)
    return sizes


def _get_nc(pattern, n_spk):
    key = (tuple(pattern), n_spk)
    if key not in _cache:
        _cache[key] = _build_nc(pattern, n_spk)
    return _cache[key]


def _split_hi_lo(x32):
    """Split fp32 array into (hi, lo) fp16 parts with x32 ~= hi + lo."""
    hi = x32.astype(np.float16)
    lo = (x32 - hi.astype(np.float32)).astype(np.float16)
    return hi, lo


def _prep_inputs(spikes, adjacency):
    flat = np.ascontiguousarray(np.asarray(spikes, dtype=np.float32).reshape(B, N))
    adj = np.asarray(adjacency, dtype=np.float32)

    # Live [ti, si] block map per core: block contributes to core i's
    # outputs iff adj[i*TSH + ti*128 : .. + 128, si*128 : (si+1)*128] has a
    # nonzero. Shipping exactly the live blocks keeps the kernel exact for
    # every input while skipping the zero blocks of conv-structured
    # adjacencies.
    bm = np.any(
        adj.reshape(NCORES, TI, W, SCHUNKS, P) != 0.0, axis=(2, 4)
    )  # [core, ti, si]

    # Align per-core block sets into one shared pattern via a per-core
    # source offset o_i (cores run one SPMD program). o_i = min(si - ti)
    # over live blocks aligns banded structures exactly.
    offs = np.zeros(NCORES, np.int64)
    pat = set()
    for i in range(NCORES):
        tis, sis = np.nonzero(bm[i])
        offs[i] = (sis - tis).min() if len(tis) else 0
        pat.update(zip(tis.tolist(), (sis - offs[i]).tolist()))
    for ti in range(TI):  # every ti needs >=1 block so PSUM gets initialized
        if not any(t == ti for t, _ in pat):
            pat.add((ti, 0))
    # si-major, ti-ascending stream order (enables merged matmuls over
    # consecutive ti sharing one stationary source chunk).
    pattern = sorted(pat, key=lambda x: (x[1], x[0]))
    n_spk = max(s for _, s in pattern) + 1

    # Stationary weights (hi/lo split), indexed by absolute source chunk.
    flatT = np.ascontiguousarray(flat.T)  # [N, B]
    fhi, flo = _split_hi_lo(flatT)
    spk_full = np.zeros((SCHUNKS, P, 32), np.float16)  # [si, p, 2*B]
    spk_full[:, :, :B] = fhi.reshape(SCHUNKS, P, B)
    spk_full[:, :, B:] = flo.reshape(SCHUNKS, P, B)

    n_blocks = len(pattern)
    group_sizes = _group_sizes(n_blocks)

    pat_ti = np.array([t for t, _ in pattern])
    pat_si_rel = np.array([s for _, s in pattern])
    in_maps = []
    for i in range(NCORES):
        o = int(offs[i])
        # Vectorized block gather: adj[t, s] viewed as [ti, tj, si, sj],
        # transposed per block to [sj, tj].
        a4 = adj[i * TSH : (i + 1) * TSH, :].reshape(TI, W, SCHUNKS, P)
        pat_si = pat_si_rel + o
        valid = (pat_si >= 0) & (pat_si < SCHUNKS)
        b32 = np.zeros((n_blocks, P, P), np.float32)  # [k, sj, tj]
        b32[valid] = a4[pat_ti[valid], :, pat_si[valid], :].transpose(0, 2, 1)
        blocks = b32.astype(np.float16)  # hi part
        blocks_lo = ((b32 - blocks.astype(np.float32)) * LO_SCALE).astype(np.float16)
        # Pack each DMA group partition-major: [p, gsz*128]; hi stream then
        # lo stream.
        parts = []
        for blk_arr in (blocks, blocks_lo):
            k0 = 0
            for gsz in group_sizes:
                parts.append(
                    np.ascontiguousarray(
                        blk_arr[k0 : k0 + gsz].transpose(1, 0, 2)
                    ).ravel()
                )
                k0 += gsz
        ablk = np.concatenate(parts)

        spk = np.zeros((n_spk, P, 32), np.float16)
        s_lo = max(0, -o)
        s_hi = min(n_spk, SCHUNKS - o)
        if s_hi > s_lo:
            spk[s_lo:s_hi] = spk_full[o + s_lo : o + s_hi]
        spk = np.ascontiguousarray(spk.transpose(1, 0, 2)).reshape(P, n_spk * 32)
        in_maps.append({"ablk": ablk, "spk": spk})
    return pattern, n_spk, in_maps


def _run(pattern, n_spk, in_maps, **kwargs):
    from concourse.bass_utils import run_bass_kernel_spmd

    return run_bass_kernel_spmd(
        _get_nc(pattern, n_spk), in_maps, core_ids=list(range(NCORES)), **kwargs
    )


def kernel(spikes, adjacency):
    pattern, n_spk, in_maps = _prep_inputs(spikes, adjacency)
    res = _run(pattern, n_spk, in_maps)
    outs = [r["o"] for r in res.results]
    # Fold spikes hi-weight rows (0:16) + lo-weight rows (16:32) and the
    # adjacency hi (cols 0:TSH) + scaled lo (cols TSH:) halves, then concat
    # the target shards.
    full = np.concatenate(
        [
            (o[:B, :TSH] + o[B:, :TSH])
            + (o[:B, TSH:] + o[B:, TSH:]) * np.float32(1.0 / LO_SCALE)
            for o in outs
        ],
        axis=1,
    )  # [B, N]
    return np.ascontiguousarray(full.reshape(B, H, W), dtype=np.float32)


# revision 52
# speedup vs baseline: 1.0273x; 1.0273x over previous
# Trainium2 Bass kernel for nn_AxonalConnections (gnn_message_passing).
#
# Computes out[B, H, W] = (spikes.reshape(B, N) @ adjacency.T).reshape(B, H, W)
# with B=16, H=W=128, N=16384 on 8 NeuronCores.
#
# Strategy (pure tensor parallelism, no collectives):
#   - Shard adjacency row-wise (target dim) across 8 cores: core i owns
#     target columns [i*2048, (i+1)*2048) of the output.
#   - The kernel is HBM/DMA-bandwidth bound, so minimize shipped bytes:
#     * input-adaptive block pruning: the host scans the adjacency at
#       [128 x 128] block granularity (source grid-row si x target grid-row
#       ti) and ships only blocks that contain nonzeros. For the conv-
#       structured adjacency this is ~112 of 2048 blocks per core (7.3 MiB
#       vs 128 MiB); for dense inputs every block ships and the kernel
#       stays exact. Per-core block sets are aligned by a per-core source
#       offset into one shared pattern so all 8 cores run the same NEFF.
#     * fp32 would stream 4x slower through the PE, so both operands ship
#       as fp16 hi/lo pairs (x = hi + lo, exact to ~2^-22): full fp32-grade
#       accuracy (~1e-6 output error) at full PE streaming rate.
#   - Spikes (tiny) are packed as the stationary operand
#     [spikes_hi | spikes_lo] (32 columns); the adjacency hi and (scaled)
#     lo block streams accumulate into separate PSUM bank sets. The host
#     folds the four partial terms and concatenates the target shards.
#   - Blocks stream si-major with merged matmuls over consecutive ti; each
#     PSUM bank finishes early and its PSUM->SBUF copy + output DMA overlap
#     the remaining matmuls.
#
# Single-queue HWDGE DMA with 8 KiB per-partition runs sustains ~410 GB/s
# (95% of the 435 GB/s SBUF-AXI fabric ceiling).

import numpy as np

B = 16
H = 128
W = 128
N = H * W            # 16384 source == target size
NCORES = 8
TSH = N // NCORES    # 2048 target columns per core
TI = TSH // W        # 16 target grid-rows per core
P = 128              # SBUF partitions / contraction tile
SCHUNKS = N // P     # 128 source chunks (== source grid-rows)
BLK_GROUP = 32       # blocks per DMA (32 * 32 KiB = 1 MiB, 8 KiB runs)
BLK = P * P          # elements per block

_cache = {}


N_WARM = 18  # PE warmup matmuls (~6 us of dummy work releases the HAM clock gate)
LO_SCALE = 1024.0  # lo-residual pre-scale (keeps fp16 lo values in normal range)


def _plan_segments(pattern, group_sizes):
    """Plan merged matmuls over the si-major block stream.

    pattern: list of (ti, si_rel), si-major then ti-ascending — the stream
    order. Blocks with consecutive ti, the same source chunk, the same PSUM
    bank, and the same DMA group merge into one matmul of N = 128*len.

    start=True is set ONLY on the first segment of each PSUM bank: on HW it
    clears has_written for the WHOLE bank, and the per-element has_written
    bit then makes every region's first write an overwrite and later writes
    accumulates — no per-region start flags needed (a later start=True
    would wipe the has_written state of sibling regions mid-accumulation).

    Returns segments: list of (k0, nblk, si_rel, ti0, start).
    """
    group_of = []
    for g, gsz in enumerate(group_sizes):
        group_of += [g] * gsz
    segments = []
    k = 0
    n = len(pattern)
    seen_banks = set()
    while k < n:
        ti0, s = pattern[k]
        ln = 1
        while (
            k + ln < n
            and pattern[k + ln] == (ti0 + ln, s)
            and (ti0 + ln) // 4 == ti0 // 4
            and group_of[k + ln] == group_of[k]
        ):
            ln += 1
        bank = ti0 // 4
        segments.append((k, ln, s, ti0, bank not in seen_banks))
        seen_banks.add(bank)
        k += ln
    return segments


def _build_nc(pattern, n_spk):
    """Build + compile the SPMD Bass program.

    pattern: list of (ti, si_rel) block coordinates in si-major stream
             order, identical for all cores. Every ti in [0, TI) appears.
    n_spk:   number of stationary source chunks shipped (max si_rel + 1).
    """
    import concourse.mybir as mybir
    import concourse.tile as tile
    from concourse import bacc

    n_blocks = len(pattern)
    group_sizes = _group_sizes(n_blocks)
    segments = _plan_segments(pattern, group_sizes)

    nc = bacc.Bacc(
        "TRN2",
        target_bir_lowering=False,
        debug=False,
        num_devices=NCORES,
    )
    # ablk: flat stream of gathered [128 x 128] fp16 blocks in `pattern`
    # order, packed per DMA-group as [p, group_blocks*128] (partition-major)
    # so every descriptor moves one contiguous 8 KiB run per partition.
    # Two halves: blocks of fp16_hi(adj), then blocks of
    # (adj - fp16_hi(adj)) * LO_SCALE — together fp32-exact to ~2^-22.
    ablk = nc.dram_tensor(
        "ablk", [2 * n_blocks * BLK], mybir.dt.float16, kind="ExternalInput"
    ).ap()
    # spk: stationary weights for the shipped source-chunk window, packed
    # [P, n_spk*32] fp16 where
    #   spk[p, k*32 + b]      = fp16_hi(spikes[b, (o_i + k)*128 + p])
    #   spk[p, k*32 + 16 + b] = fp16_lo(spikes[b, (o_i + k)*128 + p])
    # (o_i = per-core source offset; out-of-range chunks are zero).
    spk = nc.dram_tensor(
        "spk", [P, n_spk * 32], mybir.dt.float16, kind="ExternalInput"
    ).ap()
    # Output: [hi-terms | lo-terms] per target shard; host folds
    # out = (hi[0:16]+hi[16:32]) + (lo[0:16]+lo[16:32])/LO_SCALE.
    out = nc.dram_tensor(
        "o", [32, 2 * TSH], mybir.dt.float32, kind="ExternalOutput"
    ).ap()

    f32 = mybir.dt.float32
    f16 = mybir.dt.float16
    NJ = 4  # PSUM banks ([32, 512] each; 4 ti-blocks per bank)

    # Last stream index per PSUM bank (drives the drain copies).
    last_k_bank = {}
    for k, (ti, _) in enumerate(pattern):
        last_k_bank[ti // NJ] = k

    # Map stream index -> (group, local index, group start offset).
    grp_of = []
    for g, gsz in enumerate(group_sizes):
        base = len(grp_of)
        grp_of += [(g, kk - base) for kk in range(base, base + gsz)]

    with tile.TileContext(nc) as tc:
        with (
            tc.tile_pool(name="adj", bufs=min(8, 2 * len(group_sizes))) as adj_pool,
            tc.tile_pool(name="spkp", bufs=1) as spk_pool,
            tc.tile_pool(name="warm", bufs=1) as warm_pool,
            tc.tile_pool(name="psum", bufs=1, space="PSUM") as psum_pool,
            tc.tile_pool(name="outp", bufs=1) as out_pool,
        ):
            # 4 accumulator banks, shared by the hi and lo halves: the lo
            # half's first (start=True) matmul per bank re-initializes it
            # after the hi half's drain copy (Tile inserts the WAR wait).
            ps = [
                psum_pool.tile([32, NJ * P], f32, name=f"ps{j}", tag=f"ps{j}")
                for j in range(NJ)
            ]

            # PE warmup: ~6 us of dummy matmuls on a zeroed tile into a
            # dedicated scratch bank, scheduled before any real data
            # arrives. They release the HAM clock gate (cold PE runs at
            # 1.2 GHz for the first ~3.4 us of activity) so the real
            # matmuls run at 2.4 GHz from the start.
            dumt = warm_pool.tile([P, 512], f16)
            nc.gpsimd.memset(dumt[:], 0.0)
            psw = psum_pool.tile([32, 512], f32, name="psw", tag="psw")
            for _ in range(N_WARM):
                nc.tensor.matmul(
                    psw[:, :],
                    dumt[:, 0:32],
                    dumt[:, :],
                    start=True,
                    stop=True,
                    skip_group_check=True,
                )

            # Stationary weights load first on the SP ring: every matmul
            # waits on them, and on the ACT ring their packets get
            # interleaved behind the block stream (first matmul slips by
            # ~2.5 us). Serializing ~0.5 us ahead of the stream is cheaper.
            spk_t = spk_pool.tile([P, n_spk * 32], f16)
            nc.sync.dma_start(spk_t[:], spk[:])

            ot = out_pool.tile([32, 2 * TSH], f32)

            for h in range(2):  # 0 = hi stream, 1 = lo stream
                at_tiles = []
                off = h * n_blocks * BLK
                for g, gsz in enumerate(group_sizes):
                    at = adj_pool.tile([P, gsz * P], f16, name=f"at{h}_{g}", tag="at")
                    nc.sync.dma_start(
                        at[:].rearrange("p (n t) -> p n t", n=gsz),
                        ablk[off : off + gsz * BLK].rearrange(
                            "(p n t) -> p n t", p=P, t=P
                        ),
                    )
                    off += gsz * BLK
                    at_tiles.append(at)

                for k0, nblk, si_rel, ti0, start in segments:
                    g, kl = grp_of[k0]
                    j, c = divmod(ti0, NJ)
                    pj = ps[j]
                    nc.tensor.matmul(
                        pj[:, c * P : (c + nblk) * P],
                        spk_t[:, si_rel * 32 : (si_rel + 1) * 32],
                        at_tiles[g][:, kl * P : (kl + nblk) * P],
                        start=start,
                        stop=(k0 + nblk - 1 == last_k_bank[j]),
                        skip_group_check=True,
                    )
                    if k0 + nblk - 1 == last_k_bank[j]:
                        # Bank fully accumulated: drain it while the
                        # remaining banks' matmuls keep streaming. The store
                        # goes on the ACT HWDGE ring — on the (in-order) SP
                        # ring its semaphore wait would block later DMA
                        # issues behind it.
                        sl = slice(
                            (h * NJ + j) * NJ * P, (h * NJ + j + 1) * NJ * P
                        )
                        nc.vector.tensor_copy(ot[:, sl], pj[:, :])
                        nc.scalar.dma_start(out[:, sl], ot[:, sl])

    nc.compile()
    return nc


def _group_sizes(n_blocks):
    """DMA group sizes: 1 MiB groups, tapering the tail so the last
    group's matmuls + completion latency (critical path) are short.

    (A head-taper was tried and is a net loss: the PE then tracks the DMA
    stream closely and pays each group's ~1.7 us completion-receipt
    latency as a stall; with deep prefetch and a late PE start the
    completion semaphores clear before the PE reaches them.)"""
    sizes = []
    rem = n_blocks
    while rem > BLK_GROUP:
        sizes.append(BLK_GROUP)
        rem -= BLK_GROUP
    while rem > 4:
        h = max(4, rem // 2)
        sizes.append(h)
        rem -= h
    if rem:
        sizes.append(rem)
    return sizes


def _get_nc(pattern, n_spk):
    key = (tuple(pattern), n_spk)
    if key not in _cache:
        _cache[key] = _build_nc(pattern, n_spk)
    return _cache[key]


def _split_hi_lo(x32):
    """Split fp32 array into (hi, lo) fp16 parts with x32 ~= hi + lo."""
    hi = x32.astype(np.float16)
    lo = (x32 - hi.astype(np.float32)).astype(np.float16)
    return hi, lo


def _prep_inputs(spikes, adjacency):
    flat = np.ascontiguousarray(np.asarray(spikes, dtype=np.float32).reshape(B, N))
    adj = np.asarray(adjacency, dtype=np.float32)

    # Live [ti, si] block map per core: block contributes to core i's
    # outputs iff adj[i*TSH + ti*128 : .. + 128, si*128 : (si+1)*128] has a
    # nonzero. Shipping exactly the live blocks keeps the kernel exact for
    # every input while skipping the zero blocks of conv-structured
    # adjacencies.
    bm = np.any(
        adj.reshape(NCORES, TI, W, SCHUNKS, P) != 0.0, axis=(2, 4)
    )  # [core, ti, si]

    # Align per-core block sets into one shared pattern via a per-core
    # source offset o_i (cores run one SPMD program). o_i = min(si - ti)
    # over live blocks aligns banded structures exactly.
    offs = np.zeros(NCORES, np.int64)
    pat = set()
    for i in range(NCORES):
        tis, sis = np.nonzero(bm[i])
        offs[i] = (sis - tis).min() if len(tis) else 0
        pat.update(zip(tis.tolist(), (sis - offs[i]).tolist()))
    for ti in range(TI):  # every ti needs >=1 block so PSUM gets initialized
        if not any(t == ti for t, _ in pat):
            pat.add((ti, 0))
    # si-major, ti-ascending stream order (enables merged matmuls over
    # consecutive ti sharing one stationary source chunk).
    pattern = sorted(pat, key=lambda x: (x[1], x[0]))
    n_spk = max(s for _, s in pattern) + 1

    # Stationary weights (hi/lo split), indexed by absolute source chunk.
    flatT = np.ascontiguousarray(flat.T)  # [N, B]
    fhi, flo = _split_hi_lo(flatT)
    spk_full = np.zeros((SCHUNKS, P, 32), np.float16)  # [si, p, 2*B]
    spk_full[:, :, :B] = fhi.reshape(SCHUNKS, P, B)
    spk_full[:, :, B:] = flo.reshape(SCHUNKS, P, B)

    n_blocks = len(pattern)
    group_sizes = _group_sizes(n_blocks)

    pat_ti = np.array([t for t, _ in pattern])
    pat_si_rel = np.array([s for _, s in pattern])
    in_maps = []
    for i in range(NCORES):
        o = int(offs[i])
        # Vectorized block gather: adj[t, s] viewed as [ti, tj, si, sj],
        # transposed per block to [sj, tj].
        a4 = adj[i * TSH : (i + 1) * TSH, :].reshape(TI, W, SCHUNKS, P)
        pat_si = pat_si_rel + o
        valid = (pat_si >= 0) & (pat_si < SCHUNKS)
        b32 = np.zeros((n_blocks, P, P), np.float32)  # [k, sj, tj]
        b32[valid] = a4[pat_ti[valid], :, pat_si[valid], :].transpose(0, 2, 1)
        blocks = b32.astype(np.float16)  # hi part
        blocks_lo = ((b32 - blocks.astype(np.float32)) * LO_SCALE).astype(np.float16)
        # Pack each DMA group partition-major: [p, gsz*128]; hi stream then
        # lo stream.
        parts = []
        for blk_arr in (blocks, blocks_lo):
            k0 = 0
            for gsz in group_sizes:
                parts.append(
                    np.ascontiguousarray(
                        blk_arr[k0 : k0 + gsz].transpose(1, 0, 2)
                    ).ravel()
                )
                k0 += gsz
        ablk = np.concatenate(parts)

        spk = np.zeros((n_spk, P, 32), np.float16)
        s_lo = max(0, -o)
        s_hi = min(n_spk, SCHUNKS - o)
        if s_hi > s_lo:
            spk[s_lo:s_hi] = spk_full[o + s_lo : o + s_hi]
        spk = np.ascontiguousarray(spk.transpose(1, 0, 2)).reshape(P, n_spk * 32)
        in_maps.append({"ablk": ablk, "spk": spk})
    return pattern, n_spk, in_maps


def _run(pattern, n_spk, in_maps, **kwargs):
    from concourse.bass_utils import run_bass_kernel_spmd

    return run_bass_kernel_spmd(
        _get_nc(pattern, n_spk), in_maps, core_ids=list(range(NCORES)), **kwargs
    )


def kernel(spikes, adjacency):
    pattern, n_spk, in_maps = _prep_inputs(spikes, adjacency)
    res = _run(pattern, n_spk, in_maps)
    outs = [r["o"] for r in res.results]
    # Fold spikes hi-weight rows (0:16) + lo-weight rows (16:32) and the
    # adjacency hi (cols 0:TSH) + scaled lo (cols TSH:) halves, then concat
    # the target shards.
    full = np.concatenate(
        [
            (o[:B, :TSH] + o[B:, :TSH])
            + (o[:B, TSH:] + o[B:, TSH:]) * np.float32(1.0 / LO_SCALE)
            for o in outs
        ],
        axis=1,
    )  # [B, N]
    return np.ascontiguousarray(full.reshape(B, H, W), dtype=np.float32)


# revision 53
# speedup vs baseline: 1.0488x; 1.0209x over previous
# Trainium2 Bass kernel for nn_AxonalConnections (gnn_message_passing).
#
# Computes out[B, H, W] = (spikes.reshape(B, N) @ adjacency.T).reshape(B, H, W)
# with B=16, H=W=128, N=16384 on 8 NeuronCores.
#
# Strategy (pure tensor parallelism, no collectives):
#   - Shard adjacency row-wise (target dim) across 8 cores: core i owns
#     target columns [i*2048, (i+1)*2048) of the output.
#   - The kernel is HBM/DMA-bandwidth bound, so minimize shipped bytes:
#     * input-adaptive block pruning: the host scans the adjacency at
#       [128 x 128] block granularity (source grid-row si x target grid-row
#       ti) and ships only blocks that contain nonzeros. For the conv-
#       structured adjacency this is ~112 of 2048 blocks per core (7.3 MiB
#       vs 128 MiB); for dense inputs every block ships and the kernel
#       stays exact. Per-core block sets are aligned by a per-core source
#       offset into one shared pattern so all 8 cores run the same NEFF.
#     * fp32 would stream 4x slower through the PE, so both operands ship
#       as fp16 hi/lo pairs (x = hi + lo, exact to ~2^-22): full fp32-grade
#       accuracy (~1e-6 output error) at full PE streaming rate.
#   - Spikes (tiny) are packed as the stationary operand
#     [spikes_hi | spikes_lo] (32 columns); the adjacency hi and (scaled)
#     lo block streams accumulate into separate PSUM bank sets. The host
#     folds the four partial terms and concatenates the target shards.
#   - Blocks stream si-major with merged matmuls over consecutive ti; each
#     PSUM bank finishes early and its PSUM->SBUF copy + output DMA overlap
#     the remaining matmuls.
#
# Single-queue HWDGE DMA with 8 KiB per-partition runs sustains ~410 GB/s
# (95% of the 435 GB/s SBUF-AXI fabric ceiling).

import numpy as np

B = 16
H = 128
W = 128
N = H * W            # 16384 source == target size
NCORES = 8
TSH = N // NCORES    # 2048 target columns per core
TI = TSH // W        # 16 target grid-rows per core
P = 128              # SBUF partitions / contraction tile
SCHUNKS = N // P     # 128 source chunks (== source grid-rows)
BLK_GROUP = 32       # blocks per DMA (32 * 32 KiB = 1 MiB, 8 KiB runs)
BLK = P * P          # elements per block

_cache = {}


N_WARM = 18  # PE warmup matmuls (~6 us of dummy work releases the HAM clock gate)
LO_SCALE = 1024.0  # lo-residual pre-scale (keeps fp16 lo values in normal range)


def _plan_segments(pattern, group_sizes):
    """Plan merged matmuls over the si-major block stream.

    pattern: list of (ti, si_rel), si-major then ti-ascending — the stream
    order. Blocks with consecutive ti, the same source chunk, the same PSUM
    bank, and the same DMA group merge into one matmul of N = 128*len.

    start=True is set ONLY on the first segment of each PSUM bank: on HW it
    clears has_written for the WHOLE bank, and the per-element has_written
    bit then makes every region's first write an overwrite and later writes
    accumulates — no per-region start flags needed (a later start=True
    would wipe the has_written state of sibling regions mid-accumulation).

    Returns segments: list of (k0, nblk, si_rel, ti0, start).
    """
    group_of = []
    for g, gsz in enumerate(group_sizes):
        group_of += [g] * gsz
    segments = []
    k = 0
    n = len(pattern)
    seen_banks = set()
    while k < n:
        ti0, s = pattern[k]
        ln = 1
        while (
            k + ln < n
            and pattern[k + ln] == (ti0 + ln, s)
            and (ti0 + ln) // 4 == ti0 // 4
            and group_of[k + ln] == group_of[k]
        ):
            ln += 1
        bank = ti0 // 4
        segments.append((k, ln, s, ti0, bank not in seen_banks))
        seen_banks.add(bank)
        k += ln
    return segments


def _build_nc(pattern, n_spk):
    """Build + compile the SPMD Bass program.

    pattern: list of (ti, si_rel) block coordinates in si-major stream
             order, identical for all cores. Every ti in [0, TI) appears.
    n_spk:   number of stationary source chunks shipped (max si_rel + 1).
    """
    import concourse.mybir as mybir
    import concourse.tile as tile
    from concourse import bacc

    n_blocks = len(pattern)
    group_sizes = _group_sizes(n_blocks)
    segments = _plan_segments(pattern, group_sizes)

    nc = bacc.Bacc(
        "TRN2",
        target_bir_lowering=False,
        debug=False,
        num_devices=NCORES,
    )
    # ablk: flat stream of gathered [128 x 128] fp16 blocks in `pattern`
    # order, packed per DMA-group as [p, group_blocks*128] (partition-major)
    # so every descriptor moves one contiguous 8 KiB run per partition.
    # Two halves: blocks of fp16_hi(adj), then blocks of
    # (adj - fp16_hi(adj)) * LO_SCALE — together fp32-exact to ~2^-22.
    ablk = nc.dram_tensor(
        "ablk", [2 * n_blocks * BLK], mybir.dt.float16, kind="ExternalInput"
    ).ap()
    # spk: stationary weights for the shipped source-chunk window, packed
    # [P, n_spk*32] fp16 where
    #   spk[p, k*32 + b]      = fp16_hi(spikes[b, (o_i + k)*128 + p])
    #   spk[p, k*32 + 16 + b] = fp16_lo(spikes[b, (o_i + k)*128 + p])
    # (o_i = per-core source offset; out-of-range chunks are zero).
    spk = nc.dram_tensor(
        "spk", [P, n_spk * 32], mybir.dt.float16, kind="ExternalInput"
    ).ap()
    # Output: [hi-terms | lo-terms] per target shard; host folds
    # out = (hi[0:16]+hi[16:32]) + (lo[0:16]+lo[16:32])/LO_SCALE.
    out = nc.dram_tensor(
        "o", [32, 2 * TSH], mybir.dt.float32, kind="ExternalOutput"
    ).ap()

    f32 = mybir.dt.float32
    f16 = mybir.dt.float16
    NJ = 4  # PSUM banks ([32, 512] each; 4 ti-blocks per bank)

    # Last stream index per PSUM bank (drives the drain copies).
    last_k_bank = {}
    for k, (ti, _) in enumerate(pattern):
        last_k_bank[ti // NJ] = k

    # Map stream index -> (group, local index, group start offset).
    grp_of = []
    for g, gsz in enumerate(group_sizes):
        base = len(grp_of)
        grp_of += [(g, kk - base) for kk in range(base, base + gsz)]

    with tile.TileContext(nc) as tc:
        with (
            tc.tile_pool(name="adj", bufs=min(12, 2 * len(group_sizes))) as adj_pool,
            tc.tile_pool(name="spkp", bufs=1) as spk_pool,
            tc.tile_pool(name="warm", bufs=1) as warm_pool,
            tc.tile_pool(name="psum", bufs=1, space="PSUM") as psum_pool,
            tc.tile_pool(name="outp", bufs=1) as out_pool,
        ):
            # 4 accumulator banks, shared by the hi and lo halves: the lo
            # half's first (start=True) matmul per bank re-initializes it
            # after the hi half's drain copy (Tile inserts the WAR wait).
            ps = [
                psum_pool.tile([32, NJ * P], f32, name=f"ps{j}", tag=f"ps{j}")
                for j in range(NJ)
            ]

            # PE warmup: ~6 us of dummy matmuls on a zeroed tile into a
            # dedicated scratch bank, scheduled before any real data
            # arrives. They release the HAM clock gate (cold PE runs at
            # 1.2 GHz for the first ~3.4 us of activity) so the real
            # matmuls run at 2.4 GHz from the start.
            dumt = warm_pool.tile([P, 512], f16)
            nc.gpsimd.memset(dumt[:], 0.0)
            psw = psum_pool.tile([32, 512], f32, name="psw", tag="psw")
            for _ in range(N_WARM):
                nc.tensor.matmul(
                    psw[:, :],
                    dumt[:, 0:32],
                    dumt[:, :],
                    start=True,
                    stop=True,
                    skip_group_check=True,
                )

            # Stationary weights load first on the SP ring: every matmul
            # waits on them, and on the ACT ring their packets get
            # interleaved behind the block stream (first matmul slips by
            # ~2.5 us). Serializing ~0.5 us ahead of the stream is cheaper.
            spk_t = spk_pool.tile([P, n_spk * 32], f16)
            nc.sync.dma_start(spk_t[:], spk[:])

            ot = out_pool.tile([32, 2 * TSH], f32)

            for h in range(2):  # 0 = hi stream, 1 = lo stream
                at_tiles = []
                off = h * n_blocks * BLK
                for g, gsz in enumerate(group_sizes):
                    at = adj_pool.tile([P, gsz * P], f16, name=f"at{h}_{g}", tag="at")
                    nc.sync.dma_start(
                        at[:].rearrange("p (n t) -> p n t", n=gsz),
                        ablk[off : off + gsz * BLK].rearrange(
                            "(p n t) -> p n t", p=P, t=P
                        ),
                    )
                    off += gsz * BLK
                    at_tiles.append(at)

                for k0, nblk, si_rel, ti0, start in segments:
                    g, kl = grp_of[k0]
                    j, c = divmod(ti0, NJ)
                    pj = ps[j]
                    nc.tensor.matmul(
                        pj[:, c * P : (c + nblk) * P],
                        spk_t[:, si_rel * 32 : (si_rel + 1) * 32],
                        at_tiles[g][:, kl * P : (kl + nblk) * P],
                        start=start,
                        stop=(k0 + nblk - 1 == last_k_bank[j]),
                        skip_group_check=True,
                    )
                    if k0 + nblk - 1 == last_k_bank[j]:
                        # Bank fully accumulated: drain it while the
                        # remaining banks' matmuls keep streaming. The store
                        # goes on the ACT HWDGE ring — on the (in-order) SP
                        # ring its semaphore wait would block later DMA
                        # issues behind it.
                        sl = slice(
                            (h * NJ + j) * NJ * P, (h * NJ + j + 1) * NJ * P
                        )
                        nc.vector.tensor_copy(ot[:, sl], pj[:, :])
                        nc.scalar.dma_start(out[:, sl], ot[:, sl])

    nc.compile()
    return nc


def _group_sizes(n_blocks):
    """DMA group sizes: 1 MiB groups, tapering the tail so the last
    group's matmuls + completion latency (critical path) are short.

    (A head-taper was tried and is a net loss: the PE then tracks the DMA
    stream closely and pays each group's ~1.7 us completion-receipt
    latency as a stall; with deep prefetch and a late PE start the
    completion semaphores clear before the PE reaches them.)"""
    sizes = []
    rem = n_blocks
    while rem > BLK_GROUP:
        sizes.append(BLK_GROUP)
        rem -= BLK_GROUP
    while rem > 4:
        h = max(4, rem // 2)
        sizes.append(h)
        rem -= h
    if rem:
        sizes.append(rem)
    return sizes


def _get_nc(pattern, n_spk):
    key = (tuple(pattern), n_spk)
    if key not in _cache:
        _cache[key] = _build_nc(pattern, n_spk)
    return _cache[key]


def _split_hi_lo(x32):
    """Split fp32 array into (hi, lo) fp16 parts with x32 ~= hi + lo."""
    hi = x32.astype(np.float16)
    lo = (x32 - hi.astype(np.float32)).astype(np.float16)
    return hi, lo


def _prep_inputs(spikes, adjacency):
    flat = np.ascontiguousarray(np.asarray(spikes, dtype=np.float32).reshape(B, N))
    adj = np.asarray(adjacency, dtype=np.float32)

    # Live [ti, si] block map per core: block contributes to core i's
    # outputs iff adj[i*TSH + ti*128 : .. + 128, si*128 : (si+1)*128] has a
    # nonzero. Shipping exactly the live blocks keeps the kernel exact for
    # every input while skipping the zero blocks of conv-structured
    # adjacencies.
    bm = np.any(
        adj.reshape(NCORES, TI, W, SCHUNKS, P) != 0.0, axis=(2, 4)
    )  # [core, ti, si]

    # Align per-core block sets into one shared pattern via a per-core
    # source offset o_i (cores run one SPMD program). o_i = min(si - ti)
    # over live blocks aligns banded structures exactly.
    offs = np.zeros(NCORES, np.int64)
    pat = set()
    for i in range(NCORES):
        tis, sis = np.nonzero(bm[i])
        offs[i] = (sis - tis).min() if len(tis) else 0
        pat.update(zip(tis.tolist(), (sis - offs[i]).tolist()))
    for ti in range(TI):  # every ti needs >=1 block so PSUM gets initialized
        if not any(t == ti for t, _ in pat):
            pat.add((ti, 0))
    # si-major, ti-ascending stream order (enables merged matmuls over
    # consecutive ti sharing one stationary source chunk).
    pattern = sorted(pat, key=lambda x: (x[1], x[0]))
    n_spk = max(s for _, s in pattern) + 1

    # Stationary weights (hi/lo split), indexed by absolute source chunk.
    flatT = np.ascontiguousarray(flat.T)  # [N, B]
    fhi, flo = _split_hi_lo(flatT)
    spk_full = np.zeros((SCHUNKS, P, 32), np.float16)  # [si, p, 2*B]
    spk_full[:, :, :B] = fhi.reshape(SCHUNKS, P, B)
    spk_full[:, :, B:] = flo.reshape(SCHUNKS, P, B)

    n_blocks = len(pattern)
    group_sizes = _group_sizes(n_blocks)

    pat_ti = np.array([t for t, _ in pattern])
    pat_si_rel = np.array([s for _, s in pattern])
    in_maps = []
    for i in range(NCORES):
        o = int(offs[i])
        # Vectorized block gather: adj[t, s] viewed as [ti, tj, si, sj],
        # transposed per block to [sj, tj].
        a4 = adj[i * TSH : (i + 1) * TSH, :].reshape(TI, W, SCHUNKS, P)
        pat_si = pat_si_rel + o
        valid = (pat_si >= 0) & (pat_si < SCHUNKS)
        b32 = np.zeros((n_blocks, P, P), np.float32)  # [k, sj, tj]
        b32[valid] = a4[pat_ti[valid], :, pat_si[valid], :].transpose(0, 2, 1)
        blocks = b32.astype(np.float16)  # hi part
        blocks_lo = ((b32 - blocks.astype(np.float32)) * LO_SCALE).astype(np.float16)
        # Pack each DMA group partition-major: [p, gsz*128]; hi stream then
        # lo stream.
        parts = []
        for blk_arr in (blocks, blocks_lo):
            k0 = 0
            for gsz in group_sizes:
                parts.append(
                    np.ascontiguousarray(
                        blk_arr[k0 : k0 + gsz].transpose(1, 0, 2)
                    ).ravel()
                )
                k0 += gsz
        ablk = np.concatenate(parts)

        spk = np.zeros((n_spk, P, 32), np.float16)
        s_lo = max(0, -o)
        s_hi = min(n_spk, SCHUNKS - o)
        if s_hi > s_lo:
            spk[s_lo:s_hi] = spk_full[o + s_lo : o + s_hi]
        spk = np.ascontiguousarray(spk.transpose(1, 0, 2)).reshape(P, n_spk * 32)
        in_maps.append({"ablk": ablk, "spk": spk})
    return pattern, n_spk, in_maps


def _run(pattern, n_spk, in_maps, **kwargs):
    from concourse.bass_utils import run_bass_kernel_spmd

    return run_bass_kernel_spmd(
        _get_nc(pattern, n_spk), in_maps, core_ids=list(range(NCORES)), **kwargs
    )


def kernel(spikes, adjacency):
    pattern, n_spk, in_maps = _prep_inputs(spikes, adjacency)
    res = _run(pattern, n_spk, in_maps)
    outs = [r["o"] for r in res.results]
    # Fold spikes hi-weight rows (0:16) + lo-weight rows (16:32) and the
    # adjacency hi (cols 0:TSH) + scaled lo (cols TSH:) halves, then concat
    # the target shards.
    full = np.concatenate(
        [
            (o[:B, :TSH] + o[B:, :TSH])
            + (o[:B, TSH:] + o[B:, TSH:]) * np.float32(1.0 / LO_SCALE)
            for o in outs
        ],
        axis=1,
    )  # [B, N]
    return np.ascontiguousarray(full.reshape(B, H, W), dtype=np.float32)
